# revision 12
# baseline (speedup 1.0000x reference)
"""MoE BERT block kernel for 8 Trainium2 NeuronCores.

Strategy: expert parallel. The router (gate matmul + softmax + top-2) is a
~134 MFLOP computation done on the host in float64 while sharding the inputs;
token dispatch by router assignment happens during the host-side shard step.
Each of the 8 cores owns one expert's FFN weights (SBUF-resident) and runs
the dense FFN over the tokens routed to it (padded to a fixed capacity),
which is >99.9% of the FLOPs. The host then scatter-adds `w * y` per token.

Device math per core (expert e), all tokens column-major (token = free dim):
    H^T = gelu(WupT^T @ X^T + bup)      # [4096, CAP]  bf16, f32 accum
    Y^T = WdownT^T @ H^T + bdown        # [1024, CAP]  bf16 out

Speed tricks beyond the plain pipelined bf16 GEMMs:
  * Up-projection K-dims 0..255 run as ONE fp8e4 DoubleRow matmul (2x row
    rate, +13%/col) instead of two bf16 matmuls: ~11% faster up phase, and
    during tile 0 (paced by the weight DMA under the 8-core HBM burst) the
    fp8 slice also halves those K-dims' weight bytes.  fp8 quantization on a
    quarter of the up contraction measures 1.73e-2 end-to-end (gate is 2e-2;
    all-bf16 is 3.2e-3).  The fp8 operands are host-quantized with
    power-of-two scales sx=2^5 (x) and sw=2^11 (Wup); the bf16 Wup copy is
    host-scaled by sx*sw=2^16 so both paths accumulate into the same psum
    group at the same scale, removed for free via the GELU's scale=2^-16.
  * Up phase (tiles 1..4) runs ko INNER over io PAIRS alternating two psum
    banks: no ~2ns/matmul same-bank accumulation penalty, GELUs hide under
    the next pair's sweep, no 4-bank group barrier.  Tile 0 keeps ko-outer
    blocks so its matmul order matches the per-chunk weight arrival.
  * Down phase (tiles 1..4) runs io-inner sweeps over ho PAIRS (same idea);
    each pair's bias-add + output DMA hides under the next pair's sweep.
    Tile 0 keeps io-outer blocks, matching the still-arriving down weights.
  * y streams out as bf16 (halves the output DMA), biases re-added there.
"""

import os

os.environ.setdefault("MYCRO_LOCAL_CACHE", "1")

import numpy as np
import ml_dtypes

import concourse.bass as bass
import concourse.bacc as bacc
import concourse.mybir as mybir
import concourse.tile as tile
from concourse.bass_utils import run_bass_kernel_spmd

NUM_EXPERTS = 8
TOP_K = 2
H = 1024
I = 4096
P = 128
CAP = 2161  # per-expert token capacity (= max observed load; mean 2048);
# tokens beyond CAP (never expected for the reference inputs) fall back to a
# host-side numpy computation, so correctness never depends on this margin.
# Uniform tile sizes keep every matmul's streaming time (~180ns at N=432)
# above the ~53ns LDWEIGHTS floor; a small trailing tile would waste it.
TOKEN_TILES = [433, 432, 432, 432, 432]
assert sum(TOKEN_TILES) == CAP

# fp8 up-projection slice: K-dims [0, KF8) are computed by a DoubleRow fp8
# matmul. Power-of-two scales; SCALE = SX8*SW8 is also premultiplied into
# the bf16 Wup copy and divided back out in the GELU's scale argument, so
# fp8 and bf16 partials share one psum accumulation group.
KF8 = 256
KO8 = KF8 // P  # 2 bf16 ko-chunks replaced by the fp8 DoubleRow matmul
SX8 = 2.0**5  # |x| < 5.5 -> |x*sx| < 176 < 224 (TRN2 e4m3 max is 240)
SW8 = 2.0**11  # |wup| < 0.105 -> < 216
SCALE = SX8 * SW8  # 2^16
F8CLIP = 216.0  # rounds to <= 224; keeps host quantization off +-inf

BF16 = mybir.dt.bfloat16
F32 = mybir.dt.float32
F8E4 = mybir.dt.float8e4

_compiled = None  # (nc,) cache — build the Bass program once per process
last_results = None  # BassKernelResults of the most recent run (for profiling)


def _build_program():
    nc = bacc.Bacc("TRN2", target_bir_lowering=False)

    KO = H // P  # 8 contraction tiles for the up matmul (2 fp8 + 6 bf16)
    KB = KO - KO8  # bf16 ko-chunks (ko 2..7)
    IO = I // P  # 32 inter tiles (psum partition tiles up / contraction down)
    HO = H // P  # 8 output tiles for the down matmul

    # All inputs arrive pre-permuted into DMA-native per-partition layouts
    # (host packs them), so every transfer has long contiguous lines.
    xt = nc.dram_tensor("xt", [P, KB * CAP], BF16, kind="ExternalInput")
    # fp8 x pair rows (K-dims 0..255): per tile [P, 2, 448] blocks — padded
    # to 448 so each partition is one 896B contiguous DMA line AND the
    # DoubleRow rhs pair-dim stride is 16B-aligned.
    XP = 448
    xt8 = nc.dram_tensor("xt8", [P, 2 * XP * len(TOKEN_TILES)], F8E4, kind="ExternalInput")
    wup_t = nc.dram_tensor("wup_t", [P, KB * I], BF16, kind="ExternalInput")
    # fp8 Wup slice, DoubleRow layout: per io tile [P, 2, 128].
    wup8_t = nc.dram_tensor("wup8_t", [P, IO * 2 * P], F8E4, kind="ExternalInput")
    wdn_t = nc.dram_tensor("wdn_t", [P, IO * H], BF16, kind="ExternalInput")
    bup = nc.dram_tensor("bup", [P, IO], F32, kind="ExternalInput")
    bdn = nc.dram_tensor("bdn", [P, HO], F32, kind="ExternalInput")
    yt = nc.dram_tensor("yt", [H, CAP], BF16, kind="ExternalOutput")

    UPB = 4  # psum banks per tile-0 up-projection block
    DNB = 4  # psum banks per tile-0 down-projection block

    GELU_SCALE = 1.0 / SCALE
    T0 = TOKEN_TILES[0]

    with tile.TileContext(nc) as tc:
        with (
            tc.tile_pool(name="weights", bufs=1) as wpool,
            tc.tile_pool(name="xin", bufs=2) as xpool,
            tc.tile_pool(name="hmid", bufs=1) as hpool,
            tc.tile_pool(name="yout", bufs=4) as ypool,
            tc.tile_pool(name="psum_up", bufs=UPB, space="PSUM") as pu,
            tc.tile_pool(name="psum_dn", bufs=DNB, space="PSUM") as pd,
        ):
            yt_r = yt.ap().rearrange("(ho p) t -> p ho t", p=P)
            xt_ap = xt.ap()
            xt8_ap = xt8.ap()
            wup_ap = wup_t.ap()
            wup8_ap = wup8_t.ap()
            wdn_ap = wdn_t.ap()

            # DMA issue order is chosen so compute can start early: tile 0's
            # fp8 x rows + the first io-group's fp8 weights (0.37MB) gate the
            # first real matmul; each io group's remaining bf16 ko-chunks are
            # interleaved with x0's per-ko chunks so under the 8-core HBM
            # burst the per-step arrival cadence matches the ko-step compute.
            # The down weights stream in per-io chunks interleaved with tile
            # 0's up phase.
            UPG = 2 * UPB  # io tiles per tile-0 group
            x0_sb = xpool.tile([P, KB, T0], BF16, tag="x")
            x0_r = xt_ap[:, 0 : KB * T0].rearrange("p (ko t) -> p ko t", ko=KB)
            x80_sb = xpool.tile([P, 2, XP], F8E4, tag="x8")
            nc.sync.dma_start(
                x80_sb[:],
                xt8_ap[:, 0 : 2 * XP].rearrange("p (j t) -> p j t", j=2),
            )
            # Each dma_start is a ~650ns serial DMA_DIRECT2D on the sync
            # queue, so the cold ramp is ISSUE-limited: keep per-ko issues
            # only for group 0 (whose cadence paces the warm block) and ship
            # everything later as few big strided transfers.
            wup8_sb = wpool.tile([P, IO, 2, P], F8E4, tag="wup8")
            wup8_r = wup8_ap.rearrange("p (io j m) -> p io j m", io=IO, j=2)
            nc.sync.dma_start(wup8_sb[:, 0:UPG], wup8_r[:, 0:UPG])
            wup_sb = wpool.tile([P, KB, I], BF16, tag="wup")
            wup_r = wup_ap.rearrange("p (ko i) -> p ko i", ko=KB)
            for ko in range(KB):
                nc.sync.dma_start(x0_sb[:, ko], x0_r[:, ko])
                nc.sync.dma_start(
                    wup_sb[:, ko, 0 : UPG * P],
                    wup_ap[:, ko * I : ko * I + UPG * P],
                )
            bup_sb = wpool.tile([P, IO], F32, tag="bup")
            nc.sync.dma_start(bup_sb[:], bup.ap())
            bdn_sb = wpool.tile([P, HO], F32, tag="bdn")
            nc.sync.dma_start(bdn_sb[:], bdn.ap())
            nc.sync.dma_start(wup8_sb[:, UPG:], wup8_r[:, UPG:])
            for iog in range(1, IO // UPG):
                cols = slice(iog * UPG * P, (iog + 1) * UPG * P)
                nc.sync.dma_start(wup_sb[:, :, cols], wup_r[:, :, cols])
            wdn_sb = wpool.tile([P, IO, H], BF16, tag="wdn")
            wdn_r = wdn_ap.rearrange("p (io h) -> p io h", io=IO)

            # Zeroed tile for warmup / keep-alive matmuls: they have no DMA
            # dependency, so the PE starts immediately and stays busy while
            # weights stream from HBM — keeping the HAM clock gate at full
            # rate.  They accumulate 0*0 = 0 into the first live psum group,
            # which is exact, so no extra psum bank is needed.
            xw_sb = wpool.tile([P, 512], BF16, tag="warmx")
            nc.vector.memset(xw_sb[:], 0.0)

            off = 0
            for t, ntok in enumerate(TOKEN_TILES):
                if t == 0:
                    x_sb = x0_sb
                    x8_sb = x80_sb
                else:
                    x_sb = xpool.tile([P, KB, T0], BF16, tag="x")
                    nc.sync.dma_start(
                        x_sb[:, :, :ntok],
                        xt_ap[:, KB * off : KB * (off + ntok)].rearrange(
                            "p (ko t) -> p ko t", ko=KB
                        ),
                    )
                    x8_sb = xpool.tile([P, 2, XP], F8E4, tag="x8")
                    nc.sync.dma_start(
                        x8_sb[:],
                        xt8_ap[:, 2 * XP * t : 2 * XP * (t + 1)].rearrange(
                            "p (j t) -> p j t", j=2
                        ),
                    )

                # Up-projection + exact (erf) GELU: H^T tile [4096, ntok].
                h_sb = hpool.tile([P, IO, T0], BF16, tag="h")
                if t == 0:
                    # Tile 0: step-outer (DR, then ko 2..7) within a block of
                    # psum banks, so a block's matmuls can start as soon as
                    # the first weight chunk lands.  Blocks of 8 banks (the
                    # down pool is still idle) with keep-alive matmuls after
                    # each step of the first block: its pace is set by the
                    # up-weight DMA, and the fillers keep the HAM clock gate
                    # from re-throttling during the arrival gaps.
                    upb = 2 * UPB
                    for blk in range(IO // upb):
                        pss = [
                            (pu if j < UPB else pd).tile(
                                [P, T0], F32,
                                tag=("pu" if j < UPB else "pd"), name=f"pub{j}",
                            )
                            for j in range(upb)
                        ]
                        warm = blk == 0
                        if warm:
                            # PE warmup before the first data-dependent
                            # matmul: open pss[0]'s group with zeros, then
                            # bridge the PE to first-chunk arrival (~2-3us)
                            # while accumulating HAM busy time toward the
                            # 3.4us un-throttle window.
                            nc.tensor.matmul(
                                pss[0][:, :ntok], lhsT=xw_sb[:, :P],
                                rhs=xw_sb[:, :ntok], start=True, stop=False,
                            )
                            for _ in range(9):
                                nc.tensor.matmul(
                                    pss[0][:, :ntok], lhsT=xw_sb[:, :P],
                                    rhs=xw_sb[:, :ntok], start=False, stop=False,
                                )

                        def t0_step(step, j, blk=blk, pss=pss, warm=warm, ntok=ntok):
                            io = blk * upb + j
                            if step == 0:
                                nc.tensor.matmul(
                                    pss[j][:, :ntok],
                                    lhsT=wup8_sb[:, io],
                                    rhs=x80_sb[:, :, :ntok],
                                    start=not (warm and j == 0),
                                    stop=False,
                                    perf_mode=mybir.MatmulPerfMode.DoubleRow,
                                )
                            else:
                                nc.tensor.matmul(
                                    pss[j][:, :ntok],
                                    lhsT=wup_sb[:, step - 1, io * P : (io + 1) * P],
                                    rhs=x_sb[:, step - 1, :ntok],
                                    start=False,
                                    stop=(step == KB),
                                )

                        if blk == IO // upb - 1:
                            # Last tile-0 up block borrows the down pool's
                            # psum banks; close each accumulation group early
                            # (j-outer) so its GELU frees the bank while the
                            # rest of the block computes — otherwise the
                            # first down matmul stalls ~1.5us on the final
                            # four GELUs.
                            for j in range(upb):
                                for step in range(KB + 1):
                                    t0_step(step, j)
                        else:
                            for step in range(KB + 1):
                                for j in range(upb):
                                    t0_step(step, j)
                                if warm and step < KB:
                                    # Keep-alive against HBM-contention jitter.
                                    nc.tensor.matmul(
                                        pss[0][:, :ntok], lhsT=xw_sb[:, :P],
                                        rhs=xw_sb[:, :ntok], start=False, stop=False,
                                    )
                        for j in range(upb):
                            io = blk * upb + j
                            nc.scalar.activation(
                                h_sb[:, io, :ntok],
                                pss[j][:, :ntok],
                                mybir.ActivationFunctionType.Gelu,
                                bias=bup_sb[:, io : io + 1],
                                scale=GELU_SCALE,
                            )
                        # Stream the down weights while tile 0's up phase
                        # runs (one 2MB strided transfer per block).
                        nc.sync.dma_start(
                            wdn_sb[:, blk * upb : (blk + 1) * upb],
                            wdn_r[:, blk * upb : (blk + 1) * upb],
                        )
                else:
                    # Tiles 1..4: weights fully resident, so run ko INNER in
                    # io PAIRS alternating between two psum banks — one fp8
                    # DoubleRow matmul plus six bf16 matmuls per io.
                    # Alternating banks avoids the ~2ns/matmul same-bank
                    # accumulation penalty, and each pair's GELUs hide under
                    # the next pair's 2.6us sweep with no group barrier.
                    for iop in range(IO // 2):
                        pspair = [
                            pu.tile([P, T0], F32, tag="pu", name=f"pus{j}")
                            for j in range(2)
                        ]
                        for j in range(2):
                            nc.tensor.matmul(
                                pspair[j][:, :ntok],
                                lhsT=wup8_sb[:, 2 * iop + j],
                                rhs=x8_sb[:, :, :ntok],
                                start=True,
                                stop=False,
                                perf_mode=mybir.MatmulPerfMode.DoubleRow,
                            )
                        for ko in range(KB):
                            for j in range(2):
                                io = 2 * iop + j
                                nc.tensor.matmul(
                                    pspair[j][:, :ntok],
                                    lhsT=wup_sb[:, ko, io * P : (io + 1) * P],
                                    rhs=x_sb[:, ko, :ntok],
                                    start=False,
                                    stop=(ko == KB - 1),
                                )
                        for j in range(2):
                            io = 2 * iop + j
                            nc.scalar.activation(
                                h_sb[:, io, :ntok],
                                pspair[j][:, :ntok],
                                mybir.ActivationFunctionType.Gelu,
                                bias=bup_sb[:, io : io + 1],
                                scale=GELU_SCALE,
                            )

                # Down-projection + bias: Y^T tile [1024, ntok] bf16 out.
                last = t == len(TOKEN_TILES) - 1
                if t == 0:
                    # Tile 0: contraction (io) outer within a DNB-bank block:
                    # the io demand is spread across the whole phase, matching
                    # the down-weight chunks still arriving from HBM.
                    for blk in range(HO // DNB):
                        ps2 = [
                            pd.tile([P, T0], F32, tag="pd", name=f"pd{j}")
                            for j in range(DNB)
                        ]
                        for io in range(IO):
                            for j in range(DNB):
                                ho = blk * DNB + j
                                nc.tensor.matmul(
                                    ps2[j][:, :ntok],
                                    lhsT=wdn_sb[:, io, ho * P : (ho + 1) * P],
                                    rhs=h_sb[:, io, :ntok],
                                    start=(io == 0),
                                    stop=(io == IO - 1),
                                )
                        for j in range(DNB):
                            ho = blk * DNB + j
                            y_sb = ypool.tile([P, T0], BF16, tag="y")
                            nc.vector.tensor_scalar_add(
                                y_sb[:, :ntok], ps2[j][:, :ntok], bdn_sb[:, ho : ho + 1]
                            )
                            nc.sync.dma_start(
                                yt_r[:, ho, off : off + ntok], y_sb[:, :ntok]
                            )
                else:
                    # Tiles 1..4: io-inner contraction sweeps over ho PAIRS
                    # alternating two psum banks: no same-bank accumulation
                    # penalty, and each pair's bias-add + output DMA overlaps
                    # the next pair's 11.8us sweep — no group barrier and a
                    # short serial kernel tail.
                    for hop in range(HO // 2):
                        pspair = [
                            pd.tile([P, T0], F32, tag="pd", name=f"pdl{j}")
                            for j in range(2)
                        ]
                        for io in range(IO):
                            for j in range(2):
                                ho = 2 * hop + j
                                nc.tensor.matmul(
                                    pspair[j][:, :ntok],
                                    lhsT=wdn_sb[:, io, ho * P : (ho + 1) * P],
                                    rhs=h_sb[:, io, :ntok],
                                    start=(io == 0),
                                    stop=(io == IO - 1),
                                )
                        for j in range(2):
                            ho = 2 * hop + j
                            y_sb = ypool.tile([P, T0], BF16, tag="y")
                            if last and ho == HO - 1:
                                # Split the very last bias+store so the bulk
                                # DMA issues early and the final one is tiny.
                                hn = 384
                                for sl in (slice(0, hn), slice(hn, ntok)):
                                    nc.vector.tensor_scalar_add(
                                        y_sb[:, sl], pspair[j][:, sl],
                                        bdn_sb[:, ho : ho + 1],
                                    )
                                    nc.sync.dma_start(
                                        yt_r[:, ho, off + sl.start : off + sl.stop],
                                        y_sb[:, sl],
                                    )
                            else:
                                nc.vector.tensor_scalar_add(
                                    y_sb[:, :ntok], pspair[j][:, :ntok],
                                    bdn_sb[:, ho : ho + 1],
                                )
                                nc.sync.dma_start(
                                    yt_r[:, ho, off : off + ntok], y_sb[:, :ntok]
                                )
                off += ntok

    nc.compile()
    return nc


def _get_program():
    global _compiled
    if _compiled is None:
        _compiled = _build_program()
    return _compiled


def _route(X64, Wg64):
    """Replicates the reference router: softmax over gate logits, top-2."""
    T = X64.shape[0]
    logits = X64 @ Wg64.T  # [T, E]
    logits -= logits.max(axis=-1, keepdims=True)
    p = np.exp(logits)
    p /= p.sum(axis=-1, keepdims=True)
    i1 = np.argmax(p, axis=-1)
    rows = np.arange(T)
    w1 = p[rows, i1]
    p2 = p.copy()
    p2[rows, i1] = -1.0
    i2 = np.argmax(p2, axis=-1)
    w2 = p[rows, i2]
    return i1, w1, i2, w2


def _q8(a):
    """Host e4m3 quantization (values pre-scaled), saturating."""
    return np.clip(a, -F8CLIP, F8CLIP).astype(ml_dtypes.float8_e4m3)


def kernel(hidden_states, Wg, Wup, bup, Wdown, bdown):
    global last_results
    hidden_states = np.asarray(hidden_states)
    orig_shape = hidden_states.shape
    X = np.ascontiguousarray(hidden_states, dtype=np.float32).reshape(-1, H)
    T = X.shape[0]
    Wg = np.asarray(Wg, dtype=np.float32)
    Wup = np.asarray(Wup, dtype=np.float32)
    bup = np.asarray(bup, dtype=np.float32)
    Wdown = np.asarray(Wdown, dtype=np.float32)
    bdown = np.asarray(bdown, dtype=np.float32)

    # --- Router on host (float64 for a faithful top-2 ordering) ---
    i1, w1, i2, w2 = _route(X.astype(np.float64), Wg.astype(np.float64))

    # --- Dispatch: gather each expert's tokens, pad to CAP ---
    KB = H // P - KO8
    Xb = X[:, KF8:].astype(ml_dtypes.bfloat16)  # bf16 ko-chunks 2..7 only
    in_maps = []
    meta = []
    for e in range(NUM_EXPERTS):
        sel1 = np.nonzero(i1 == e)[0]
        sel2 = np.nonzero(i2 == e)[0]
        idx = np.concatenate([sel1, sel2])
        wts = np.concatenate([w1[sel1], w2[sel2]])
        n = idx.size
        overflow = None
        if n > CAP:
            # Never expected for the reference inputs (max load 2161); kept as
            # a correctness safety net: spill tokens are computed on the host.
            overflow = (idx[CAP:], wts[CAP:])
            idx, wts = idx[:CAP], wts[:CAP]
            n = CAP
        idx_pad = np.concatenate([idx, np.zeros(CAP - n, dtype=idx.dtype)])
        # Pack into the kernel's DMA-native per-partition layouts:
        #  xt:  per token tile [P, KB, ntok] blocks (K-dims 256..1023, bf16)
        #  xt8: per token tile [P, 2, ntok] blocks (K-dims 0..255, fp8 *SX8)
        #  wup: Wup[:, 256:].T * SCALE as [P, KB*I]
        #  wup8: [P, IO, 2, 128] (fp8, *SW8);  wdn: Wdown.T as [P, IO*H]
        xt_full = Xb[idx_pad].T.reshape(KB, P, CAP)  # [KB, P, CAP]
        x8_full = _q8(X[idx_pad, :KF8] * SX8).reshape(CAP, 2, P)  # [CAP, 2, P]
        XP = 448  # per-tile fp8 block padded to 448 tokens (one 896B line)
        blocks = []
        blocks8 = []
        o = 0
        for ntok in TOKEN_TILES:
            blocks.append(xt_full[:, :, o : o + ntok].transpose(1, 0, 2).reshape(P, -1))
            b8 = np.zeros((P, 2, XP), dtype=ml_dtypes.float8_e4m3)
            b8[:, :, :ntok] = x8_full[o : o + ntok].transpose(2, 1, 0)
            blocks8.append(b8.reshape(P, -1))
            o += ntok
        xt_dev = np.concatenate(blocks, axis=1)
        xt8_dev = np.concatenate(blocks8, axis=1)
        wup_dev = (
            (Wup[e][:, KF8:] * SCALE).astype(ml_dtypes.bfloat16)
            .T.reshape(KB, P, I).transpose(1, 0, 2).reshape(P, -1)
        )
        # wup8[p, io, j, m] = Wup[io*128+m, 128*j+p] * SW8
        wup8_dev = (
            _q8(Wup[e][:, :KF8] * SW8)
            .reshape(I // P, P, 2, P)  # [io, m, j, p]
            .transpose(3, 0, 2, 1)
            .reshape(P, -1)
        )
        wdn_dev = (
            Wdown[e].astype(ml_dtypes.bfloat16).T.reshape(I // P, P, H)
            .transpose(1, 0, 2).reshape(P, -1)
        )
        in_maps.append(
            {
                "xt": np.ascontiguousarray(xt_dev),
                "xt8": np.ascontiguousarray(xt8_dev),
                "wup_t": np.ascontiguousarray(wup_dev),
                "wup8_t": np.ascontiguousarray(wup8_dev),
                "wdn_t": np.ascontiguousarray(wdn_dev),
                "bup": np.ascontiguousarray(bup[e].reshape(I // P, P).T),
                "bdn": np.ascontiguousarray(bdown[e].reshape(H // P, P).T),
            }
        )
        meta.append((idx, wts, overflow))

    # --- Run the Bass kernel on all 8 cores ---
    nc = _get_program()
    last_results = run_bass_kernel_spmd(nc, in_maps, core_ids=list(range(8)))

    # --- Combine: out[token] += w * y ---
    out = np.zeros((T, H), dtype=np.float32)
    for e in range(NUM_EXPERTS):
        idx, wts, overflow = meta[e]
        yt_full = np.asarray(last_results.results[e]["yt"])  # [H, CAP] bf16
        Y = yt_full.T[: idx.size].astype(np.float32)  # [n, H]
        out[idx] += wts[:, None].astype(np.float32) * Y
        if overflow is not None:
            oidx, owts = overflow
            from scipy.special import erf

            xo = X[oidx]
            h_in = xo @ Wup[e].T + bup[e]
            h = 0.5 * h_in * (1.0 + erf(h_in / np.sqrt(2.0)))
            yo = h @ Wdown[e].T + bdown[e]
            out[oidx] += owts[:, None].astype(np.float32) * yo
    return out.reshape(orig_shape)


# revision 14
# speedup vs baseline: 1.0182x; 1.0182x over previous
"""MoE BERT block kernel for 8 Trainium2 NeuronCores.

Strategy: expert parallel. The router (gate matmul + softmax + top-2) is a
~134 MFLOP computation done on the host in float64 while sharding the inputs;
token dispatch by router assignment happens during the host-side shard step.
Each of the 8 cores owns one expert's FFN weights (SBUF-resident) and runs
the dense FFN over the tokens routed to it (padded to a fixed capacity),
which is >99.9% of the FLOPs. The host then scatter-adds `w * y` per token.

Device math per core (expert e), all tokens column-major (token = free dim):
    H^T = gelu(WupT^T @ X^T + bup)      # [4096, CAP]  bf16, f32 accum
    Y^T = WdownT^T @ H^T + bdown        # [1024, CAP]  bf16 out

Speed tricks beyond the plain pipelined bf16 GEMMs:
  * Up-projection K-dims 0..255 run as ONE fp8e4 DoubleRow matmul (2x row
    rate, +13%/col) instead of two bf16 matmuls: ~11% faster up phase, and
    during tile 0 (paced by the weight DMA under the 8-core HBM burst) the
    fp8 slice also halves those K-dims' weight bytes.  fp8 quantization on a
    quarter of the up contraction measures 1.73e-2 end-to-end (gate is 2e-2;
    all-bf16 is 3.2e-3).  The fp8 operands are host-quantized with
    power-of-two scales sx=2^5 (x) and sw=2^11 (Wup); the bf16 Wup copy is
    host-scaled by sx*sw=2^16 so both paths accumulate into the same psum
    group at the same scale, removed for free via the GELU's scale=2^-16.
  * Up phase (tiles 1..4) runs ko INNER over io PAIRS alternating two psum
    banks: no ~2ns/matmul same-bank accumulation penalty, GELUs hide under
    the next pair's sweep, no 4-bank group barrier.  Tile 0 keeps ko-outer
    blocks so its matmul order matches the per-chunk weight arrival.
  * Down phase (tiles 1..4) runs io-inner sweeps over ho PAIRS (same idea);
    each pair's bias-add + output DMA hides under the next pair's sweep.
    Tile 0 keeps io-outer blocks, matching the still-arriving down weights.
  * y streams out as bf16 (halves the output DMA), biases re-added there.
"""

import os

os.environ.setdefault("MYCRO_LOCAL_CACHE", "1")

import numpy as np
import ml_dtypes

import concourse.bass as bass
import concourse.bacc as bacc
import concourse.mybir as mybir
import concourse.tile as tile
from concourse.bass_utils import run_bass_kernel_spmd

NUM_EXPERTS = 8
TOP_K = 2
H = 1024
I = 4096
P = 128
CAP = 2161  # per-expert token capacity (= max observed load; mean 2048);
# tokens beyond CAP (never expected for the reference inputs) fall back to a
# host-side numpy computation, so correctness never depends on this margin.
# Uniform tile sizes keep every matmul's streaming time (~180ns at N=432)
# above the ~53ns LDWEIGHTS floor; a small trailing tile would waste it.
TOKEN_TILES = [433, 432, 432, 432, 432]
assert sum(TOKEN_TILES) == CAP

# fp8 up-projection slice: K-dims [0, KF8) are computed by a DoubleRow fp8
# matmul. Power-of-two scales; SCALE = SX8*SW8 is also premultiplied into
# the bf16 Wup copy and divided back out in the GELU's scale argument, so
# fp8 and bf16 partials share one psum accumulation group.
KF8 = 256
KO8 = KF8 // P  # 2 bf16 ko-chunks replaced by the fp8 DoubleRow matmul
SX8 = 2.0**5  # |x| < 5.5 -> |x*sx| < 176 < 224 (TRN2 e4m3 max is 240)
SW8 = 2.0**11  # |wup| < 0.105 -> < 216
SCALE = SX8 * SW8  # 2^16
F8CLIP = 216.0  # rounds to <= 224; keeps host quantization off +-inf

BF16 = mybir.dt.bfloat16
F32 = mybir.dt.float32
F8E4 = mybir.dt.float8e4

_compiled = None  # (nc,) cache — build the Bass program once per process
last_results = None  # BassKernelResults of the most recent run (for profiling)


def _build_program():
    nc = bacc.Bacc("TRN2", target_bir_lowering=False)

    KO = H // P  # 8 contraction tiles for the up matmul (2 fp8 + 6 bf16)
    KB = KO - KO8  # bf16 ko-chunks (ko 2..7)
    IO = I // P  # 32 inter tiles (psum partition tiles up / contraction down)
    HO = H // P  # 8 output tiles for the down matmul

    # All inputs arrive pre-permuted into DMA-native per-partition layouts
    # (host packs them), so every transfer has long contiguous lines.
    xt = nc.dram_tensor("xt", [P, KB * CAP], BF16, kind="ExternalInput")
    # fp8 x pair rows (K-dims 0..255): per tile [P, 2, 448] blocks — padded
    # to 448 so each partition is one 896B contiguous DMA line AND the
    # DoubleRow rhs pair-dim stride is 16B-aligned.
    XP = 448
    xt8 = nc.dram_tensor("xt8", [P, 2 * XP * len(TOKEN_TILES)], F8E4, kind="ExternalInput")
    wup_t = nc.dram_tensor("wup_t", [P, KB * I], BF16, kind="ExternalInput")
    # fp8 Wup slice, DoubleRow layout: per io tile [P, 2, 128].
    wup8_t = nc.dram_tensor("wup8_t", [P, IO * 2 * P], F8E4, kind="ExternalInput")
    wdn_t = nc.dram_tensor("wdn_t", [P, IO * H], BF16, kind="ExternalInput")
    bup = nc.dram_tensor("bup", [P, IO], F32, kind="ExternalInput")
    bdn = nc.dram_tensor("bdn", [P, HO], F32, kind="ExternalInput")
    yt = nc.dram_tensor("yt", [H, CAP], BF16, kind="ExternalOutput")

    UPB = 4  # psum banks per tile-0 up-projection block
    DNB = 4  # psum banks per tile-0 down-projection block

    GELU_SCALE = 1.0 / SCALE
    T0 = TOKEN_TILES[0]

    with tile.TileContext(nc) as tc:
        with (
            tc.tile_pool(name="weights", bufs=1) as wpool,
            tc.tile_pool(name="xin", bufs=2) as xpool,
            tc.tile_pool(name="hmid", bufs=1) as hpool,
            tc.tile_pool(name="yout", bufs=4) as ypool,
            tc.tile_pool(name="psum_up", bufs=UPB, space="PSUM") as pu,
            tc.tile_pool(name="psum_dn", bufs=DNB, space="PSUM") as pd,
        ):
            yt_r = yt.ap().rearrange("(ho p) t -> p ho t", p=P)
            xt_ap = xt.ap()
            xt8_ap = xt8.ap()
            wup_ap = wup_t.ap()
            wup8_ap = wup8_t.ap()
            wdn_ap = wdn_t.ap()

            # DMA issue order is chosen so compute can start early: tile 0's
            # fp8 x rows + the first io-group's fp8 weights (0.37MB) gate the
            # first real matmul; each io group's remaining bf16 ko-chunks are
            # interleaved with x0's per-ko chunks so under the 8-core HBM
            # burst the per-step arrival cadence matches the ko-step compute.
            # The down weights stream in per-io chunks interleaved with tile
            # 0's up phase.
            UPG = 2 * UPB  # io tiles per tile-0 group
            x0_sb = xpool.tile([P, KB, T0], BF16, tag="x")
            x0_r = xt_ap[:, 0 : KB * T0].rearrange("p (ko t) -> p ko t", ko=KB)
            x80_sb = xpool.tile([P, 2, XP], F8E4, tag="x8")
            nc.sync.dma_start(
                x80_sb[:],
                xt8_ap[:, 0 : 2 * XP].rearrange("p (j t) -> p j t", j=2),
            )
            # Each dma_start is a ~650ns serial DMA_DIRECT2D on the sync
            # queue with an ~8-deep in-flight window keyed on COMPLETIONS,
            # so both many tiny transfers (issue-bound) and few huge ones
            # (window-bound) lose; ~0.26MB chunks in consumption order win.
            wup8_sb = wpool.tile([P, IO, 2, P], F8E4, tag="wup8")
            wup8_r = wup8_ap.rearrange("p (io j m) -> p io j m", io=IO, j=2)
            nc.sync.dma_start(wup8_sb[:, 0:UPG], wup8_r[:, 0:UPG])
            wup_sb = wpool.tile([P, KB, I], BF16, tag="wup")
            for ko in range(KB):
                nc.sync.dma_start(x0_sb[:, ko], x0_r[:, ko])
                nc.sync.dma_start(
                    wup_sb[:, ko, 0 : UPG * P],
                    wup_ap[:, ko * I : ko * I + UPG * P],
                )
            bup_sb = wpool.tile([P, IO], F32, tag="bup")
            nc.sync.dma_start(bup_sb[:], bup.ap())
            bdn_sb = wpool.tile([P, HO], F32, tag="bdn")
            nc.sync.dma_start(bdn_sb[:], bdn.ap())
            for iog in range(1, IO // UPG):
                nc.sync.dma_start(
                    wup8_sb[:, iog * UPG : (iog + 1) * UPG],
                    wup8_r[:, iog * UPG : (iog + 1) * UPG],
                )
                for ko in range(KB):
                    nc.sync.dma_start(
                        wup_sb[:, ko, iog * UPG * P : (iog + 1) * UPG * P],
                        wup_ap[:, ko * I + iog * UPG * P : ko * I + (iog + 1) * UPG * P],
                    )
            wdn_sb = wpool.tile([P, IO, H], BF16, tag="wdn")

            # Zeroed tile for warmup / keep-alive matmuls: they have no DMA
            # dependency, so the PE starts immediately and stays busy while
            # weights stream from HBM — keeping the HAM clock gate at full
            # rate.  They accumulate 0*0 = 0 into the first live psum group,
            # which is exact, so no extra psum bank is needed.
            xw_sb = wpool.tile([P, 512], BF16, tag="warmx")
            nc.vector.memset(xw_sb[:], 0.0)

            off = 0
            for t, ntok in enumerate(TOKEN_TILES):
                if t == 0:
                    x_sb = x0_sb
                    x8_sb = x80_sb
                else:
                    x_sb = xpool.tile([P, KB, T0], BF16, tag="x")
                    nc.sync.dma_start(
                        x_sb[:, :, :ntok],
                        xt_ap[:, KB * off : KB * (off + ntok)].rearrange(
                            "p (ko t) -> p ko t", ko=KB
                        ),
                    )
                    x8_sb = xpool.tile([P, 2, XP], F8E4, tag="x8")
                    nc.sync.dma_start(
                        x8_sb[:],
                        xt8_ap[:, 2 * XP * t : 2 * XP * (t + 1)].rearrange(
                            "p (j t) -> p j t", j=2
                        ),
                    )

                # Up-projection + exact (erf) GELU: H^T tile [4096, ntok].
                h_sb = hpool.tile([P, IO, T0], BF16, tag="h")
                if t == 0:
                    # Tile 0: step-outer (DR, then ko 2..7) within a block of
                    # psum banks, so a block's matmuls can start as soon as
                    # the first weight chunk lands.  Blocks of 8 banks (the
                    # down pool is still idle) with keep-alive matmuls after
                    # each step of the first block: its pace is set by the
                    # up-weight DMA, and the fillers keep the HAM clock gate
                    # from re-throttling during the arrival gaps.
                    upb = 2 * UPB
                    for blk in range(IO // upb):
                        pss = [
                            (pu if j < UPB else pd).tile(
                                [P, T0], F32,
                                tag=("pu" if j < UPB else "pd"), name=f"pub{j}",
                            )
                            for j in range(upb)
                        ]
                        warm = blk == 0
                        if warm:
                            # PE warmup before the first data-dependent
                            # matmul: open pss[0]'s group with zeros, then
                            # bridge the PE to first-chunk arrival (~2-3us)
                            # while accumulating HAM busy time toward the
                            # 3.4us un-throttle window.
                            nc.tensor.matmul(
                                pss[0][:, :ntok], lhsT=xw_sb[:, :P],
                                rhs=xw_sb[:, :ntok], start=True, stop=False,
                            )
                            for _ in range(9):
                                nc.tensor.matmul(
                                    pss[0][:, :ntok], lhsT=xw_sb[:, :P],
                                    rhs=xw_sb[:, :ntok], start=False, stop=False,
                                )

                        def t0_step(step, j, blk=blk, pss=pss, warm=warm, ntok=ntok):
                            io = blk * upb + j
                            if step == 0:
                                nc.tensor.matmul(
                                    pss[j][:, :ntok],
                                    lhsT=wup8_sb[:, io],
                                    rhs=x80_sb[:, :, :ntok],
                                    start=not (warm and j == 0),
                                    stop=False,
                                    perf_mode=mybir.MatmulPerfMode.DoubleRow,
                                )
                            else:
                                nc.tensor.matmul(
                                    pss[j][:, :ntok],
                                    lhsT=wup_sb[:, step - 1, io * P : (io + 1) * P],
                                    rhs=x_sb[:, step - 1, :ntok],
                                    start=False,
                                    stop=(step == KB),
                                )

                        if blk == IO // upb - 1:
                            # Last tile-0 up block borrows the down pool's
                            # psum banks; close each accumulation group early
                            # (j-outer) so its GELU frees the bank while the
                            # rest of the block computes — otherwise the
                            # first down matmul stalls ~1.5us on the final
                            # four GELUs.
                            for j in range(upb):
                                for step in range(KB + 1):
                                    t0_step(step, j)
                        else:
                            for step in range(KB + 1):
                                for j in range(upb):
                                    t0_step(step, j)
                                if warm and step < KB:
                                    # Keep-alive against HBM-contention jitter.
                                    nc.tensor.matmul(
                                        pss[0][:, :ntok], lhsT=xw_sb[:, :P],
                                        rhs=xw_sb[:, :ntok], start=False, stop=False,
                                    )
                        for j in range(upb):
                            io = blk * upb + j
                            nc.scalar.activation(
                                h_sb[:, io, :ntok],
                                pss[j][:, :ntok],
                                mybir.ActivationFunctionType.Gelu,
                                bias=bup_sb[:, io : io + 1],
                                scale=GELU_SCALE,
                            )
                        # Stream the down weights while tile 0's up phase runs.
                        for io in range(blk * upb, (blk + 1) * upb):
                            nc.sync.dma_start(
                                wdn_sb[:, io], wdn_ap[:, io * H : (io + 1) * H]
                            )
                else:
                    # Tiles 1..4: weights fully resident, so run ko INNER in
                    # io PAIRS alternating between two psum banks — one fp8
                    # DoubleRow matmul plus six bf16 matmuls per io.
                    # Alternating banks avoids the ~2ns/matmul same-bank
                    # accumulation penalty, and each pair's GELUs hide under
                    # the next pair's 2.6us sweep with no group barrier.
                    for iop in range(IO // 2):
                        pspair = [
                            pu.tile([P, T0], F32, tag="pu", name=f"pus{j}")
                            for j in range(2)
                        ]
                        for j in range(2):
                            nc.tensor.matmul(
                                pspair[j][:, :ntok],
                                lhsT=wup8_sb[:, 2 * iop + j],
                                rhs=x8_sb[:, :, :ntok],
                                start=True,
                                stop=False,
                                perf_mode=mybir.MatmulPerfMode.DoubleRow,
                            )
                        for ko in range(KB):
                            for j in range(2):
                                io = 2 * iop + j
                                nc.tensor.matmul(
                                    pspair[j][:, :ntok],
                                    lhsT=wup_sb[:, ko, io * P : (io + 1) * P],
                                    rhs=x_sb[:, ko, :ntok],
                                    start=False,
                                    stop=(ko == KB - 1),
                                )
                        for j in range(2):
                            io = 2 * iop + j
                            nc.scalar.activation(
                                h_sb[:, io, :ntok],
                                pspair[j][:, :ntok],
                                mybir.ActivationFunctionType.Gelu,
                                bias=bup_sb[:, io : io + 1],
                                scale=GELU_SCALE,
                            )

                # Down-projection + bias: Y^T tile [1024, ntok] bf16 out.
                last = t == len(TOKEN_TILES) - 1
                if t == 0:
                    # Tile 0: contraction (io) outer within a DNB-bank block:
                    # the io demand is spread across the whole phase, matching
                    # the down-weight chunks still arriving from HBM.
                    for blk in range(HO // DNB):
                        ps2 = [
                            pd.tile([P, T0], F32, tag="pd", name=f"pd{j}")
                            for j in range(DNB)
                        ]
                        for io in range(IO):
                            for j in range(DNB):
                                ho = blk * DNB + j
                                nc.tensor.matmul(
                                    ps2[j][:, :ntok],
                                    lhsT=wdn_sb[:, io, ho * P : (ho + 1) * P],
                                    rhs=h_sb[:, io, :ntok],
                                    start=(io == 0),
                                    stop=(io == IO - 1),
                                )
                        for j in range(DNB):
                            ho = blk * DNB + j
                            y_sb = ypool.tile([P, T0], BF16, tag="y")
                            nc.vector.tensor_scalar_add(
                                y_sb[:, :ntok], ps2[j][:, :ntok], bdn_sb[:, ho : ho + 1]
                            )
                            nc.sync.dma_start(
                                yt_r[:, ho, off : off + ntok], y_sb[:, :ntok]
                            )
                else:
                    # Tiles 1..4: io-inner contraction sweeps over ho PAIRS
                    # alternating two psum banks: no same-bank accumulation
                    # penalty, and each pair's bias-add + output DMA overlaps
                    # the next pair's 11.8us sweep — no group barrier and a
                    # short serial kernel tail.
                    for hop in range(HO // 2):
                        pspair = [
                            pd.tile([P, T0], F32, tag="pd", name=f"pdl{j}")
                            for j in range(2)
                        ]
                        for io in range(IO):
                            for j in range(2):
                                ho = 2 * hop + j
                                nc.tensor.matmul(
                                    pspair[j][:, :ntok],
                                    lhsT=wdn_sb[:, io, ho * P : (ho + 1) * P],
                                    rhs=h_sb[:, io, :ntok],
                                    start=(io == 0),
                                    stop=(io == IO - 1),
                                )
                        for j in range(2):
                            ho = 2 * hop + j
                            y_sb = ypool.tile([P, T0], BF16, tag="y")
                            if last and ho == HO - 1:
                                # Split the very last bias+store so the bulk
                                # DMA issues early and the final one is tiny.
                                hn = 384
                                for sl in (slice(0, hn), slice(hn, ntok)):
                                    nc.vector.tensor_scalar_add(
                                        y_sb[:, sl], pspair[j][:, sl],
                                        bdn_sb[:, ho : ho + 1],
                                    )
                                    nc.sync.dma_start(
                                        yt_r[:, ho, off + sl.start : off + sl.stop],
                                        y_sb[:, sl],
                                    )
                            else:
                                nc.vector.tensor_scalar_add(
                                    y_sb[:, :ntok], pspair[j][:, :ntok],
                                    bdn_sb[:, ho : ho + 1],
                                )
                                nc.sync.dma_start(
                                    yt_r[:, ho, off : off + ntok], y_sb[:, :ntok]
                                )
                off += ntok

    nc.compile()
    return nc


def _get_program():
    global _compiled
    if _compiled is None:
        _compiled = _build_program()
    return _compiled


def _route(X64, Wg64):
    """Replicates the reference router: softmax over gate logits, top-2."""
    T = X64.shape[0]
    logits = X64 @ Wg64.T  # [T, E]
    logits -= logits.max(axis=-1, keepdims=True)
    p = np.exp(logits)
    p /= p.sum(axis=-1, keepdims=True)
    i1 = np.argmax(p, axis=-1)
    rows = np.arange(T)
    w1 = p[rows, i1]
    p2 = p.copy()
    p2[rows, i1] = -1.0
    i2 = np.argmax(p2, axis=-1)
    w2 = p[rows, i2]
    return i1, w1, i2, w2


def _q8(a):
    """Host e4m3 quantization (values pre-scaled), saturating."""
    return np.clip(a, -F8CLIP, F8CLIP).astype(ml_dtypes.float8_e4m3)


def kernel(hidden_states, Wg, Wup, bup, Wdown, bdown):
    global last_results
    hidden_states = np.asarray(hidden_states)
    orig_shape = hidden_states.shape
    X = np.ascontiguousarray(hidden_states, dtype=np.float32).reshape(-1, H)
    T = X.shape[0]
    Wg = np.asarray(Wg, dtype=np.float32)
    Wup = np.asarray(Wup, dtype=np.float32)
    bup = np.asarray(bup, dtype=np.float32)
    Wdown = np.asarray(Wdown, dtype=np.float32)
    bdown = np.asarray(bdown, dtype=np.float32)

    # --- Router on host (float64 for a faithful top-2 ordering) ---
    i1, w1, i2, w2 = _route(X.astype(np.float64), Wg.astype(np.float64))

    # --- Dispatch: gather each expert's tokens, pad to CAP ---
    KB = H // P - KO8
    Xb = X[:, KF8:].astype(ml_dtypes.bfloat16)  # bf16 ko-chunks 2..7 only
    in_maps = []
    meta = []
    for e in range(NUM_EXPERTS):
        sel1 = np.nonzero(i1 == e)[0]
        sel2 = np.nonzero(i2 == e)[0]
        idx = np.concatenate([sel1, sel2])
        wts = np.concatenate([w1[sel1], w2[sel2]])
        n = idx.size
        overflow = None
        if n > CAP:
            # Never expected for the reference inputs (max load 2161); kept as
            # a correctness safety net: spill tokens are computed on the host.
            overflow = (idx[CAP:], wts[CAP:])
            idx, wts = idx[:CAP], wts[:CAP]
            n = CAP
        idx_pad = np.concatenate([idx, np.zeros(CAP - n, dtype=idx.dtype)])
        # Pack into the kernel's DMA-native per-partition layouts:
        #  xt:  per token tile [P, KB, ntok] blocks (K-dims 256..1023, bf16)
        #  xt8: per token tile [P, 2, ntok] blocks (K-dims 0..255, fp8 *SX8)
        #  wup: Wup[:, 256:].T * SCALE as [P, KB*I]
        #  wup8: [P, IO, 2, 128] (fp8, *SW8);  wdn: Wdown.T as [P, IO*H]
        xt_full = Xb[idx_pad].T.reshape(KB, P, CAP)  # [KB, P, CAP]
        x8_full = _q8(X[idx_pad, :KF8] * SX8).reshape(CAP, 2, P)  # [CAP, 2, P]
        XP = 448  # per-tile fp8 block padded to 448 tokens (one 896B line)
        blocks = []
        blocks8 = []
        o = 0
        for ntok in TOKEN_TILES:
            blocks.append(xt_full[:, :, o : o + ntok].transpose(1, 0, 2).reshape(P, -1))
            b8 = np.zeros((P, 2, XP), dtype=ml_dtypes.float8_e4m3)
            b8[:, :, :ntok] = x8_full[o : o + ntok].transpose(2, 1, 0)
            blocks8.append(b8.reshape(P, -1))
            o += ntok
        xt_dev = np.concatenate(blocks, axis=1)
        xt8_dev = np.concatenate(blocks8, axis=1)
        wup_dev = (
            (Wup[e][:, KF8:] * SCALE).astype(ml_dtypes.bfloat16)
            .T.reshape(KB, P, I).transpose(1, 0, 2).reshape(P, -1)
        )
        # wup8[p, io, j, m] = Wup[io*128+m, 128*j+p] * SW8
        wup8_dev = (
            _q8(Wup[e][:, :KF8] * SW8)
            .reshape(I // P, P, 2, P)  # [io, m, j, p]
            .transpose(3, 0, 2, 1)
            .reshape(P, -1)
        )
        wdn_dev = (
            Wdown[e].astype(ml_dtypes.bfloat16).T.reshape(I // P, P, H)
            .transpose(1, 0, 2).reshape(P, -1)
        )
        in_maps.append(
            {
                "xt": np.ascontiguousarray(xt_dev),
                "xt8": np.ascontiguousarray(xt8_dev),
                "wup_t": np.ascontiguousarray(wup_dev),
                "wup8_t": np.ascontiguousarray(wup8_dev),
                "wdn_t": np.ascontiguousarray(wdn_dev),
                "bup": np.ascontiguousarray(bup[e].reshape(I // P, P).T),
                "bdn": np.ascontiguousarray(bdown[e].reshape(H // P, P).T),
            }
        )
        meta.append((idx, wts, overflow))

    # --- Run the Bass kernel on all 8 cores ---
    nc = _get_program()
    last_results = run_bass_kernel_spmd(nc, in_maps, core_ids=list(range(8)))

    # --- Combine: out[token] += w * y ---
    out = np.zeros((T, H), dtype=np.float32)
    for e in range(NUM_EXPERTS):
        idx, wts, overflow = meta[e]
        yt_full = np.asarray(last_results.results[e]["yt"])  # [H, CAP] bf16
        Y = yt_full.T[: idx.size].astype(np.float32)  # [n, H]
        out[idx] += wts[:, None].astype(np.float32) * Y
        if overflow is not None:
            oidx, owts = overflow
            from scipy.special import erf

            xo = X[oidx]
            h_in = xo @ Wup[e].T + bup[e]
            h = 0.5 * h_in * (1.0 + erf(h_in / np.sqrt(2.0)))
            yo = h @ Wdown[e].T + bdown[e]
            out[oidx] += owts[:, None].astype(np.float32) * yo
    return out.reshape(orig_shape)
